# revision 10
# baseline (speedup 1.0000x reference)
"""Trainium2 (Bass/Tile) kernel for BatchMarginRankingLoss over a PyG-style
batch of B=64 graphs x 1024 edges.

Math
----
reference: for every graph, over all unordered slot pairs i<j:
    loss_ij = relu(sign(y_i - y_j) * (x_j - x_i))
then per-graph mean over the C = n(n-1)/2 pairs, then mean over graphs.

The full n x n pair-loss matrix L[p, f] = relu(sign(y_p - y_f) * (x_f - x_p))
is symmetric with zero diagonal, so  sum_{i<j} loss = 0.5 * sum_{p,f} L.
Per element, with w = x_f - x_p and H = [y_f > y_p]:
    L[p, f] = relu(w) - H * w
(check: y_p > y_f -> H=0 -> relu(w); y_p < y_f -> H=1 -> relu(w)-w = relu(-w);
 p == f -> w=0 -> 0).  So each 128x1024 tile needs only
    1) w   = Xrow - x_col            (vector tensor_scalar, bf16)
    2) gs  = (Yrow > y_col) * w      (vector scalar_tensor_tensor, accum -> sum_f H*w)
    3) rs  = relu(w)                 (scalar activation, accum -> sum_f relu(w))
and the final scalar is  sum(rcols - gcols) * 1/(2*C*B).

Sharding: 8 graphs per core (batch dim), contiguous slices of the edge
arrays; each core reduces its graphs to one partial that already includes
the 1/(2*C*B) scaling, so the host just sums the 8 partials.
"""
import numpy as np
from contextlib import ExitStack

import concourse.bass as bass
import concourse.bacc as bacc
import concourse.tile as tile
from concourse import mybir
from concourse.alu_op_type import AluOpType
from concourse.bass import _add_dep_helper
from concourse.bass_utils import run_bass_kernel_spmd

B = 64            # graphs in the batch
PMAX = 1024       # edges per graph
N_CORES = 8
B_LOC = B // N_CORES            # 8 graphs per core
E_LOC = B_LOC * PMAX            # 8192 edges per core
CHUNKS = PMAX // 128            # 8 partition-chunks per graph
N_TILES = B_LOC * CHUNKS        # 64 tiles per core
PAIR_COUNT = PMAX * (PMAX - 1) // 2
SCALE = 1.0 / (2.0 * PAIR_COUNT * B)

F32 = mybir.dt.float32
BF16 = mybir.dt.bfloat16


def build_nc(reps: int = 1) -> bacc.Bacc:
    """reps>1 unrolls the whole compute `reps` times (same result; used to
    measure per-iteration HW time by wall-clock slope)."""
    nc = bacc.Bacc()
    x_ext = nc.declare_dram_parameter("x", [E_LOC], F32, isOutput=False)
    y_ext = nc.declare_dram_parameter("y", [E_LOC], F32, isOutput=False)
    out_ext = nc.declare_dram_parameter("out", [1, 1], F32, isOutput=True)

    with tile.TileContext(nc) as tc, ExitStack() as ctx:
        singles = ctx.enter_context(tc.tile_pool(name="singles", bufs=1))
        rows = ctx.enter_context(tc.tile_pool(name="rows", bufs=2))
        work = ctx.enter_context(tc.tile_pool(name="work", bufs=4))
        scratch = ctx.enter_context(tc.tile_pool(name="scratch", bufs=2))
        psum = ctx.enter_context(tc.tile_pool(name="psum", bufs=1, space="PSUM"))
        dram = ctx.enter_context(tc.tile_pool(name="dram", bufs=1, space="DRAM"))

        # ---- prologue: bf16 copies of x/y staged to DRAM scratch (source for
        # the per-graph broadcast-row DMAs)
        xbf_dram = dram.tile([B_LOC, PMAX], BF16)
        ybf_dram = dram.tile([B_LOC, PMAX], BF16)

        def stage_bf16(ext, bf_dram, tag):
            g8_f = singles.tile([B_LOC, PMAX], F32, tag=f"{tag}_g8f")
            nc.sync.dma_start(g8_f[:], ext[:].rearrange("(g n) -> g n", g=B_LOC))
            g8 = singles.tile([B_LOC, PMAX], BF16, tag=f"{tag}_g8")
            nc.vector.tensor_copy(g8[:], g8_f[:])
            nc.sync.dma_start(bf_dram[:], g8[:])

        stage_bf16(x_ext, xbf_dram, "x")
        stage_bf16(y_ext, ybf_dram, "y")

        # per-partition scalar columns, one [128, CHUNKS] f32 tile per graph:
        # xcol_g[p, r] = x[g*PMAX + 128*r + p]  (strided 4KB DMA from DRAM)
        xcols, ycols = [], []
        for g in range(B_LOC):
            xc = singles.tile([128, CHUNKS], F32, tag=f"xcol{g}")
            nc.sync.dma_start(
                xc[:], x_ext[g * PMAX:(g + 1) * PMAX].rearrange("(r p) -> p r", p=128))
            yc = singles.tile([128, CHUNKS], F32, tag=f"ycol{g}")
            nc.sync.dma_start(
                yc[:], y_ext[g * PMAX:(g + 1) * PMAX].rearrange("(r p) -> p r", p=128))
            xcols.append(xc)
            ycols.append(yc)

        rcols = singles.tile([128, N_TILES], F32)
        ones_bf = singles.tile([128, 1], BF16)
        nc.vector.memset(ones_bf[:], 1.0)
        # PSUM accumulator for sum_p of all gs tiles: [1, PMAX] f32
        psA = psum.tile([1, PMAX], F32)

        # ---- main loop: 8 graphs x 8 chunks (x reps)
        for rep in range(reps):
            for g in range(B_LOC):
                Xrow = rows.tile([128, PMAX], BF16, tag="Xrow")
                nc.sync.dma_start(Xrow[:],
                                  xbf_dram[g:g + 1, :].partition_broadcast(128))
                Yrow = rows.tile([128, PMAX], BF16, tag="Yrow")
                nc.sync.dma_start(Yrow[:],
                                  ybf_dram[g:g + 1, :].partition_broadcast(128))
                for r in range(CHUNKS):
                    t = g * CHUNKS + r
                    w = work.tile([128, PMAX], BF16, tag="w")
                    nc.vector.tensor_scalar(
                        w[:], Xrow[:], xcols[g][:, r:r + 1], None,
                        AluOpType.subtract)
                    h2 = work.tile([128, PMAX], BF16, tag="h2")
                    nc.vector.tensor_scalar(
                        h2[:], Yrow[:], ycols[g][:, r:r + 1], None,
                        AluOpType.is_gt)
                    gs = scratch.tile([128, PMAX], BF16, tag="gs")
                    nc.vector.tensor_tensor(gs[:], h2[:], w[:], AluOpType.mult)
                    # accumulate sum over partitions of gs into psA (all tiles)
                    for half in range(2):
                        nc.tensor.matmul(
                            psA[:, half * 512:(half + 1) * 512], ones_bf[:],
                            gs[:, half * 512:(half + 1) * 512],
                            start=(t == 0), stop=(t == N_TILES - 1))
                    rs = scratch.tile([128, PMAX], BF16, tag="rs")
                    nc.scalar.activation(
                        rs[:], w[:], mybir.ActivationFunctionType.Relu,
                        accum_out=rcols[:, t:t + 1])

        # ---- epilogue: total = (sum(rcols) - sum(psA)) * SCALE
        dsum = singles.tile([128, 1], F32)
        nc.vector.tensor_reduce(dsum[:], rcols[:], mybir.AxisListType.X, AluOpType.add)
        ones = singles.tile([128, 1], F32)
        nc.vector.memset(ones[:], 1.0)
        ps = psum.tile([1, 1], F32)
        nc.tensor.matmul(ps[:], ones[:], dsum[:], start=True, stop=True)
        gtot = singles.tile([1, 1], F32)
        nc.vector.tensor_reduce(gtot[:], psA[:], mybir.AxisListType.X, AluOpType.add)
        rtot = singles.tile([1, 1], F32)
        nc.scalar.activation(rtot[:], ps[:], mybir.ActivationFunctionType.Identity)
        diff = singles.tile([1, 1], F32)
        nc.vector.tensor_tensor(diff[:], rtot[:], gtot[:], AluOpType.subtract)
        outsb = singles.tile([1, 1], F32)
        nc.scalar.activation(outsb[:], diff[:], mybir.ActivationFunctionType.Identity,
                             scale=float(SCALE))
        nc.sync.dma_start(out_ext[:], outsb[:])

    nc.finalize()
    return nc


class _Runner:
    """Persistent compiled executor for the SPMD bass program: traces and
    compiles the jit once, then each call is just a dispatch. Mirrors
    concourse.bass2jax.run_bass_via_pjrt's multi-core branch."""

    def __init__(self, nc):
        import jax
        from jax.experimental.shard_map import shard_map
        from jax.sharding import Mesh, PartitionSpec
        from concourse import bass2jax

        bass2jax.install_neuronx_cc_hook()
        self.nc = nc
        in_names, out_names, out_avals, zero_outs = [], [], [], []
        partition_name = (nc.partition_id_tensor.name
                          if nc.partition_id_tensor else None)
        for alloc in nc.m.functions[0].allocations:
            if not isinstance(alloc, mybir.MemoryLocationSet):
                continue
            name = alloc.memorylocations[0].name
            if alloc.kind == "ExternalInput":
                if name != partition_name:
                    in_names.append(name)
            elif alloc.kind == "ExternalOutput":
                shape = tuple(alloc.tensor_shape)
                dtype = mybir.dt.np(alloc.dtype)
                out_names.append(name)
                out_avals.append(jax.core.ShapedArray(shape, dtype))
                zero_outs.append(np.zeros(shape, dtype))
        n_params = len(in_names)
        n_outs = len(out_avals)
        all_in_names = list(in_names) + list(out_names)
        if partition_name is not None:
            all_in_names.append(partition_name)
        self.in_names = in_names
        self.out_names = out_names
        self.zero_outs = zero_outs
        donate = tuple(range(n_params, n_params + n_outs))

        def _body(*args):
            operands = list(args)
            if partition_name is not None:
                operands.append(bass2jax.partition_id_tensor())
            outs = bass2jax._bass_exec_p.bind(
                *operands,
                out_avals=tuple(out_avals),
                in_names=tuple(all_in_names),
                out_names=tuple(out_names),
                lowering_input_output_aliases=(),
                sim_require_finite=True,
                sim_require_nnan=True,
                nc=nc,
            )
            return tuple(outs)

        devices = jax.devices()[:N_CORES]
        assert len(devices) == N_CORES
        mesh = Mesh(np.asarray(devices), ("core",))
        in_specs = (PartitionSpec("core"),) * (n_params + n_outs)
        out_specs = (PartitionSpec("core"),) * n_outs
        self._jit = jax.jit(
            shard_map(_body, mesh=mesh, in_specs=in_specs, out_specs=out_specs,
                      check_rep=False),
            donate_argnums=donate, keep_unused=True)

    def __call__(self, in_maps):
        import jax
        concat_in = [
            np.concatenate([np.asarray(in_maps[c][k]) for c in range(N_CORES)],
                           axis=0)
            for k in self.in_names
        ]
        zeros = [np.concatenate([z] * N_CORES, axis=0) for z in self.zero_outs]
        outs = self._jit(*concat_in, *zeros)
        outs = [np.asarray(o) for o in jax.block_until_ready(outs)]
        res = []
        for c in range(N_CORES):
            m = {}
            for i, name in enumerate(self.out_names):
                n0 = self.zero_outs[i].shape[0]
                m[name] = outs[i][c * n0:(c + 1) * n0]
            res.append(m)
        return res


_RUNNERS: dict = {}


def get_runner(reps: int = 1) -> _Runner:
    if reps not in _RUNNERS:
        _RUNNERS[reps] = _Runner(build_nc(reps))
    return _RUNNERS[reps]


def kernel(outputs: np.ndarray, y: np.ndarray, edges_batch: np.ndarray) -> np.ndarray:
    outputs = np.ascontiguousarray(np.asarray(outputs, dtype=np.float32))
    y = np.ascontiguousarray(np.asarray(y, dtype=np.float32))
    eb = np.asarray(edges_batch)
    assert outputs.shape == (B * PMAX,) and y.shape == (B * PMAX,)
    # this kernel is specialized to the PyG-style equal-sized-graph batch the
    # problem generates: edges_batch == repeat(arange(B), PMAX)
    expected_eb = np.repeat(np.arange(B, dtype=eb.dtype), PMAX)
    assert np.array_equal(eb, expected_eb), "kernel requires equal-sized graphs"

    in_maps = [
        {"x": outputs[i * E_LOC:(i + 1) * E_LOC], "y": y[i * E_LOC:(i + 1) * E_LOC]}
        for i in range(N_CORES)
    ]
    res = get_runner(1)(in_maps)
    total = np.float64(0.0)
    for i in range(N_CORES):
        total += np.float64(res[i]["out"][0, 0])
    return np.asarray(total, dtype=np.float32)
